# revision 30
# baseline (speedup 1.0000x reference)
"""BiLSTM-CRF loss kernel for 8 Trainium2 NeuronCores.

Sharding: phase 1 (embedding-projection + LSTM + emission GEMM) runs on
8 cores = 2 directions x 4 batch-quarters (16 examples/core, 512 steps).
The backward direction is realized by feeding time-reversed embeddings
through the same forward program. Phase 2 (CRF forward algorithm) runs
on 8 cores = 8 examples/core. Host glues the phases (pair-sum of the
two fc halves, gold-path score gathers, final logsumexp/mean).
"""

import numpy as np
import ml_dtypes

import concourse.bacc as bacc
import concourse.mybir as mybir
from concourse import tile
from concourse.bass_utils import run_bass_kernel_spmd

V, T, E, H = 50000, 32, 256, 512
B, S = 64, 512
BC = 16            # batch per core, phase 1
B2 = 8             # batch per core, phase 2
NCORES = 8
CHUNK = 32         # LSTM steps per projection chunk
NCHUNK = S // CHUNK
RENORM = 8         # CRF renormalization cadence

AF = mybir.ActivationFunctionType
F32 = mybir.dt.float32
BF16 = mybir.dt.bfloat16
ALU = mybir.AluOpType

_built = {}
_last_exec_ns = []


def _run_spmd(nc, in_maps):
    """run_bass_kernel_spmd; records per-launch time in _last_exec_ns.

    With KTRACE=1 tries the NTFF profile path for true HW time; if the
    hook is unavailable falls back to wall time around the launch."""
    import os, time
    if os.environ.get("KTRACE"):
        try:
            r = run_bass_kernel_spmd(nc, in_maps,
                                     core_ids=list(range(NCORES)), trace=True)
            if r.exec_time_ns:
                _last_exec_ns.append(r.exec_time_ns)
                return r
        except Exception as e:  # trace plumbing unavailable -> plain run
            print("KTRACE unavailable:", repr(e))
        t0 = time.time()
        r = run_bass_kernel_spmd(nc, in_maps, core_ids=list(range(NCORES)))
        _last_exec_ns.append(int((time.time() - t0) * 1e9))
        return r
    return run_bass_kernel_spmd(nc, in_maps, core_ids=list(range(NCORES)))


def _new_nc():
    return bacc.Bacc("TRN2", target_bir_lowering=False, debug=False,
                     num_devices=NCORES)


def build_phase1_v2(nsteps=S):
    """LSTM + emissions, latency-optimized recurrence.

    Gate chunk order along 4H is (g, i, f, o) x 4 psum-column chunks of 128.
    Input projections AND biases are matmul-accumulated into the same psum
    region the recurrence accumulates into, so gates never touch the DVE
    before activation. Per-step critical path: gate matmuls -> tanh(g) ->
    sigmoid(i,f) -> DVE [i*tanh(g)|f*c] -> DVE add -> tanh(c) -> DVE o*tc.
    """
    nc = _new_nc()
    GS = 4                  # steps per projection group
    ng = nsteps // GS
    eT = nc.dram_tensor("eT", [2, 128, nsteps * BC], BF16, kind="ExternalInput")
    wih = nc.dram_tensor("wihT", [2, 128, 4 * H], BF16, kind="ExternalInput")
    whh = nc.dram_tensor("whhT", [4, 128, 4 * H], BF16, kind="ExternalInput")
    fcw = nc.dram_tensor("fcwT", [4, 128, T], BF16, kind="ExternalInput")
    bia = nc.dram_tensor("biaT", [1, 4 * H], BF16, kind="ExternalInput")
    emo = nc.dram_tensor("emT", [T, nsteps * BC], F32, kind="ExternalOutput")

    with tile.TileContext(nc) as tc:
        with (
            tc.tile_pool(name="weights", bufs=1) as wpool,
            tc.tile_pool(name="state", bufs=1) as spool,
            tc.tile_pool(name="gact", bufs=3) as apool,
            tc.tile_pool(name="tmp", bufs=3) as tpool,
            tc.tile_pool(name="psgg", bufs=2, space="PSUM") as pgg,
            tc.tile_pool(name="psgf", bufs=2, space="PSUM") as pgf,
            tc.tile_pool(name="pse", bufs=2, space="PSUM") as pepool,
        ):
            wih_s = wpool.tile([128, 2, 4 * H], BF16, tag="wih")
            whh_s = wpool.tile([128, 4, 4 * H], BF16, tag="whh")
            fcw_s = wpool.tile([128, 4, T], BF16, tag="fcw")
            bia_s = wpool.tile([1, 4 * H], BF16, tag="bia")
            ones_s = wpool.tile([1, GS * BC], BF16, tag="ones")
            eT_s = spool.tile([128, 2, nsteps * BC], BF16, tag="eT")
            hbuf = spool.tile([128, 4, nsteps * BC], BF16, tag="hbuf")
            hzero = spool.tile([128, BC], BF16, tag="hzero")
            # gc[:, 0:4] = tanh(g) scratch, gc[:, 4:8] = persistent cell c —
            # adjacent so one DVE op computes [i*tanh(g) | f*c]
            gc = spool.tile([128, 8, BC], F32, tag="gc")

            for k in range(2):
                nc.gpsimd.dma_start(wih_s[:, k, :], wih[k, :, :])
                nc.gpsimd.dma_start(eT_s[:, k, :], eT[k, :, :])
            for k in range(4):
                nc.gpsimd.dma_start(whh_s[:, k, :], whh[k, :, :])
                nc.gpsimd.dma_start(fcw_s[:, k, :], fcw[k, :, :])
            nc.gpsimd.dma_start(bia_s[:], bia[:, :])
            nc.vector.memset(ones_s[:], 1.0)
            nc.vector.memset(hzero[:], 0.0)
            nc.vector.memset(gc[:], 0.0)

            gtiles = {}

            def gslice(g, m):
                """psum slice for gate chunk m: g-gates in their own tile so
                tanh(g) does not wait on the whole step's matmuls."""
                tg, ti = gtiles[g]
                return tg[:, m] if m < 4 else ti[:, m - 4]

            def proj_quarter(g, q):
                """Project gate chunks 4q..4q+3 of group g (xp + bias)."""
                if q == 0:
                    # padded to whole psum banks so each buffer owns its
                    # 2KB zero regions (start=True marks a full region)
                    tg = pgg.tile([128, 4, GS, BC], F32, tag="gg", name="gg",
                                  padded_shape=[128, 8, GS, BC])
                    ti = pgf.tile([128, 12, GS, BC], F32, tag="gifo",
                                  name="gifo", padded_shape=[128, 16, GS, BC])
                    gtiles[g] = (tg, ti)
                cs = slice(g * GS * BC, (g + 1) * GS * BC)
                for m in range(4 * q, 4 * q + 4):
                    ms = slice(m * 128, (m + 1) * 128)
                    gp_m = gslice(g, m)
                    nc.tensor.matmul(gp_m, wih_s[:, 0, ms],
                                     eT_s[:, 0, cs], start=(m in (0, 4, 12)),
                                     stop=False, skip_group_check=True)
                    nc.tensor.matmul(gp_m, wih_s[:, 1, ms],
                                     eT_s[:, 1, cs], start=False, stop=False,
                                     skip_group_check=True)
                    nc.tensor.matmul(gp_m, bia_s[:, ms], ones_s[:],
                                     start=False, stop=False,
                                     skip_group_check=True)

            EMC = min(32, nsteps)

            def em_chunk(ch):
                pe = pepool.tile([T, EMC * BC], F32, tag="pse")
                cs = slice(ch * EMC * BC, (ch + 1) * EMC * BC)
                for hk in range(4):
                    nc.tensor.matmul(pe[:], fcw_s[:, hk, :], hbuf[:, hk, cs],
                                     start=(hk == 0), stop=(hk == 3))
                est = tpool.tile([T, EMC * BC], F32, tag="est")
                nc.scalar.copy(est[:], pe[:])
                nc.gpsimd.dma_start(emo[:, cs], est[:])

            for q in range(4):
                proj_quarter(0, q)

            for t in range(nsteps):
                g, s = divmod(t, GS)
                tg, ti = gtiles[g]
                for m in range(16):
                    ms = slice(m * 128, (m + 1) * 128)
                    for hk in range(4):
                        rhs = (hzero[:] if t == 0 else
                               hbuf[:, hk, (t - 1) * BC:t * BC])
                        nc.tensor.matmul(
                            gslice(g, m)[:, s], whh_s[:, hk, ms], rhs,
                            start=False, stop=(hk == 3 and s == GS - 1),
                            skip_group_check=True)
                if g + 1 < ng:
                    proj_quarter(g + 1, s)
                if t % EMC == 0 and t >= EMC:
                    em_chunk(t // EMC - 1)

                # gate chunk order (g, i, f, o). tanh(g) lands next to c so
                # one DVE op forms [i*tanh(g) | f*c]; sigma covers i,f AND o
                # in one in-place psum op (o consumed from psum by h-mul)
                nc.scalar.activation(gc[:, 0:4], tg[:, :, s], AF.Tanh)
                nc.scalar.activation(ti[:, 0:8, s], ti[:, 0:8, s],
                                     AF.Sigmoid)
                t12 = tpool.tile([128, 8, BC], F32, tag="t12")
                nc.vector.tensor_mul(t12[:], ti[:, 0:8, s], gc[:])
                nc.vector.tensor_add(gc[:, 4:8], t12[:, 0:4], t12[:, 4:8])
                # sigma(o) emitted after the c-chain: engine-counting sync
                # means earlier DVE ops would otherwise wait on it
                oact = apool.tile([128, 4, BC], BF16, tag="oact")
                nc.scalar.activation(oact[:], ti[:, 8:12, s], AF.Sigmoid)
                tch = tpool.tile([128, 4, BC], F32, tag="tch")
                nc.scalar.activation(tch[:], gc[:, 4:8], AF.Tanh)
                nc.vector.tensor_mul(hbuf[:, :, t * BC:(t + 1) * BC],
                                     tch[:], oact[:])

            em_chunk(nsteps // EMC - 1)
    nc.compile()
    return nc


def build_phase2(nsteps=S):
    """Linear-space CRF forward pass.

    ea_t = (exp(trans).T @ ea_{t-1}) * exp(em_t), renormalized every RENORM
    steps by a stale column-sum reciprocal (the exact factors applied are
    shipped to the host in rO, so any positive scale is mathematically
    exact). Per-step chain is matmul -> one DVE multiply.
    """
    nc = _new_nc()
    em = nc.dram_tensor("emT2", [T, nsteps * B2], F32, kind="ExternalInput")
    ex = nc.dram_tensor("expT", [T, T + 1], F32, kind="ExternalInput")
    ao = nc.dram_tensor("alphaO", [T, B2], F32, kind="ExternalOutput")
    nren = (nsteps - 2) // RENORM          # r_k applied at t=(k+1)*RENORM
    ro = nc.dram_tensor("rO", [1, max(nren, 1) * B2], F32,
                        kind="ExternalOutput")

    with tile.TileContext(nc) as tc:
        with (
            tc.tile_pool(name="sb", bufs=1) as sb,
            tc.tile_pool(name="ea", bufs=3) as eap,
            tc.tile_pool(name="ps", bufs=2, space="PSUM") as pp,
            tc.tile_pool(name="psb", bufs=2, space="PSUM") as pb,
        ):
            em_s = sb.tile([T, nsteps * B2], F32, tag="em")
            eem = sb.tile([T, nsteps * B2], F32, tag="eem")
            ex_s = sb.tile([T, T + 1], F32, tag="ex")
            onesc = sb.tile([1, T], F32, tag="onesc")
            rst = sb.tile([1, max(nren, 1) * B2], F32, tag="rst")
            nc.gpsimd.dma_start(ex_s[:], ex[:, :])
            nc.vector.memset(onesc[:], 1.0)
            if nren < 1:
                nc.vector.memset(rst[:], 1.0)

            # bulk exp of emissions, chunked so the loop can start early
            NCH = 8
            cw = nsteps * B2 // NCH
            for ch in range(NCH):
                cs = slice(ch * cw, (ch + 1) * cw)
                nc.gpsimd.dma_start(em_s[:, cs], em[:, cs])
                nc.scalar.activation(eem[:, cs], em_s[:, cs], AF.Exp)

            ea_prev = eem[:, 0:B2]
            rbc = None
            for t in range(1, nsteps):
                pt = pp.tile([T + 1, B2], F32, tag="pt")
                nc.tensor.matmul(pt[:], ex_s[:], ea_prev, start=True,
                                 stop=True)
                ea = eap.tile([T, B2], F32, tag="ea")
                es = slice(t * B2, (t + 1) * B2)
                nc.vector.tensor_mul(ea[:], pt[0:T, :], eem[:, es])
                if t % RENORM == 0 and t >= RENORM and rbc is not None:
                    nc.vector.tensor_mul(ea[:], ea[:], rbc[:])
                k = t // RENORM
                if t % RENORM == RENORM - 2 and k < nren:
                    rs = slice(k * B2, (k + 1) * B2)
                    nc.vector.reciprocal(rst[:, rs], pt[T:T + 1, :])
                if t % RENORM == RENORM - 1 and k < nren:
                    rs = slice(k * B2, (k + 1) * B2)
                    rbc = pb.tile([T, B2], F32, tag="rbc")
                    nc.tensor.matmul(rbc[:], onesc[:], rst[:, rs],
                                     start=True, stop=True)
                ea_prev = ea[:]

            alpha = sb.tile([T, B2], F32, tag="alpha")
            nc.scalar.copy(alpha[:], ea_prev)
            nc.gpsimd.dma_start(ao[:, :], alpha[:])
            nc.gpsimd.dma_start(ro[:, :], rst[:])
    nc.compile()
    return nc


def _bf16(a):
    return np.ascontiguousarray(a.astype(ml_dtypes.bfloat16))


def _perm_figo(w):
    """Reorder gate rows from reference (i,f,g,o) to kernel (g,i,f,o)."""
    return np.concatenate([w[2 * H:3 * H], w[0:H], w[H:2 * H], w[3 * H:]], 0)


def _prep_core_p1(e_sbe, wih_d, whh_d, b_d, fcw_half):
    """e_sbe: [16, S, E] embedded (already time-reversed for bwd cores)."""
    eT = _bf16(e_sbe.transpose(2, 1, 0).reshape(2, 128, S * BC))
    wihT = _bf16(_perm_figo(wih_d).T.reshape(2, 128, 4 * H))
    whhT = _bf16(_perm_figo(whh_d).T.reshape(4, 128, 4 * H))
    fcwT = _bf16(fcw_half.T.reshape(4, 128, T))
    biaT = _bf16(_perm_figo(b_d).reshape(1, 4 * H))
    return {"eT": eT, "wihT": wihT, "whhT": whhT, "fcwT": fcwT,
            "biaT": biaT}


def kernel(emb, w_ih_f, w_hh_f, b_f, w_ih_b, w_hh_b, b_b, fc_w, fc_b,
           start_trans, end_trans, trans, x, tags):
    emb = np.asarray(emb, np.float32)
    fc_w = np.asarray(fc_w, np.float32)
    fc_b = np.asarray(fc_b, np.float32)
    start_trans = np.asarray(start_trans, np.float32)
    end_trans = np.asarray(end_trans, np.float32)
    trans = np.asarray(trans, np.float32)
    x = np.asarray(x).astype(np.int64)
    tags_np = np.asarray(tags).astype(np.int64)

    if "p1" not in _built:
        _built["p1"] = build_phase1_v2()
        _built["p2"] = build_phase2()
    nc1, nc2 = _built["p1"], _built["p2"]

    in_maps = []
    for core in range(NCORES):
        d = core // 4          # 0 = forward, 1 = backward
        q = core % 4
        xs = x[q * BC:(q + 1) * BC]
        if d == 1:
            xs = xs[:, ::-1]
        e = emb[xs]            # [16, S, E]
        if d == 0:
            in_maps.append(_prep_core_p1(e, np.asarray(w_ih_f, np.float32),
                                         np.asarray(w_hh_f, np.float32),
                                         np.asarray(b_f, np.float32),
                                         fc_w[:, :H]))
        else:
            in_maps.append(_prep_core_p1(e, np.asarray(w_ih_b, np.float32),
                                         np.asarray(w_hh_b, np.float32),
                                         np.asarray(b_b, np.float32),
                                         fc_w[:, H:]))
    _last_exec_ns.clear()
    r1 = _run_spmd(nc1, in_maps)

    em = np.empty((S, B, T), np.float32)
    for q in range(4):
        emf = r1.results[q]["emT"].reshape(T, S, BC).transpose(1, 2, 0)
        emb_r = r1.results[4 + q]["emT"].reshape(T, S, BC).transpose(1, 2, 0)
        em[:, q * BC:(q + 1) * BC, :] = emf + emb_r[::-1] + fc_b
    em[0] += start_trans

    # gold-path (numerator) score from device emissions + tag lookups
    tags_t = tags_np.T
    emit = np.take_along_axis(em, tags_t[:, :, None], axis=2)[..., 0].sum(0)
    tr = trans[tags_t[:-1], tags_t[1:]].sum(0)
    num = emit + tr + end_trans[tags_t[-1]]
    # (start_trans already folded into em[0])

    expT = np.concatenate([np.exp(trans), np.ones((T, 1), np.float32)],
                          axis=1).astype(np.float32)
    in_maps2 = []
    for core in range(NCORES):
        emc = em[:, core * B2:(core + 1) * B2, :]       # [S, 8, T]
        emT2 = np.ascontiguousarray(
            emc.transpose(2, 0, 1).reshape(T, S * B2).astype(np.float32))
        in_maps2.append({"emT2": emT2, "expT": expT})
    r2 = _run_spmd(nc2, in_maps2)

    nren = (S - 2) // RENORM
    eend = np.exp(end_trans.astype(np.float64))
    den = np.empty(B, np.float64)
    for core in range(NCORES):
        a = r2.results[core]["alphaO"].astype(np.float64)   # [T, 8] linear
        r = r2.results[core]["rO"].reshape(nren, B2).astype(np.float64)
        w = (a * eend[:, None]).sum(0)
        den[core * B2:(core + 1) * B2] = np.log(w) - np.log(r).sum(0)

    llh = num - den
    return np.float32(-llh.mean())

